# revision 22
# baseline (speedup 1.0000x reference)
"""Trainium2 Bass kernel for nn_DynamicWeightAttention.

Reference computation (per token t = (bt, n, h)):
    fused = concat(dyn[bt,n,h,:], static[n,h,:])            # C=32
    normed = LayerNorm(fused; gamma, beta, eps=1e-4)
    hmid   = tanh(normed @ w1 + b1)                         # HID=64
    score  = hmid @ w2 + b2                                 # scalar
    out[bt,n,:] = softmax over h of score                   # H=16

Strategy (8 NeuronCores, data-sharded over N: core c owns n in [32c, 32c+32)):
  - All LayerNorm affine, bias, mean-subtraction and static-feature terms
    fold host-side into per-n 128x128 weight blocks (sel-row trick): only
    per-token invstd is computed on device.
  - Stats via bn_stats (mean+var of even/odd f-halves in one DVE pass) +
    a short recombination chain + bit-trick rsqrt. No ACT square, no
    1x tensor_reduce passes.
  - Prescale dyn by inv on GPSIMD into packed slabs; xbar DMA transpose
    makes feature-major rhs tiles; mm1: 2 bf16 K=128 matmuls per n ->
    h_pre for 4 tokens/column in PSUM; tanh on ACT (FD=1024).
  - mm2: 8 concurrent 64x8 row/col-tiled matmuls (tile_position) fold w2;
    scores land in 2 PSUM banks as [8=h-slot, (nl,bt)] per token-parity.
  - Softmax: DVE cast PSUM->bf16, xbar transpose to token-major, exp on
    ACT over valid slots only, DVE segment reduce + reciprocal + multiply,
    bf16 DMA out (host casts to f32 and unpermutes the h slots).
"""
import os

import numpy as np
import ml_dtypes

import concourse.bacc as bacc
import concourse.mybir as mybir
from concourse.ap import AP as BassAP
from concourse.tile import TileContext
from concourse.bass_utils import run_bass_kernel_spmd

F32 = mybir.dt.float32
BF16 = mybir.dt.bfloat16
FP16 = mybir.dt.float16
U32 = mybir.dt.uint32
AT = mybir.AluOpType
AFT = mybir.ActivationFunctionType

B_T, N, H, PD, PS, HID = 1024, 256, 16, 16, 16, 64
NCORES = 8
NC_N = N // NCORES          # 32 n's per core
EPS = 1e-4
MAGIC = float(0x5F3759DF)

_cached = {}


def _host_prep(dynamic_features, static_features, ln_gamma, ln_beta, w1, b1, w2, b2):
    """Fold LN affine + mean-subtraction + static features into per-core
    packed weights; cast the dynamic stream to bf16."""
    g = np.asarray(ln_gamma, np.float32)
    be = np.asarray(ln_beta, np.float32)
    w1 = np.asarray(w1, np.float32)
    b1 = np.asarray(b1, np.float32)
    w2v = np.asarray(w2, np.float32).reshape(HID)
    st = np.asarray(static_features, np.float32)

    w1g = w1 * g[:, None]                      # [32, 64]
    cw = w1g.sum(0)                            # [64]
    w1dp = w1g[:PD] - cw[None, :] / 32.0       # [16, 64] dyn part, mean-folded
    w1s = w1g[PD:]                             # [16, 64] static part
    b1p = b1 + be @ w1                         # [64]

    # mm2 stationary: [128 = 2 parity x 64 d, 8 s, 8 jcol]
    # s = half*4 + v ; col j = 2v + half ; same pattern for both parities.
    m2 = np.zeros((128, 8, 8), np.float32)
    for half in range(2):
        for v in range(4):
            s = half * 4 + v
            j = 2 * v + half
            m2[0:64, s, j] = w2v
            m2[64:128, s, j] = w2v

    dyn = np.asarray(dynamic_features, np.float32)

    per_core = []
    for c in range(NCORES):
        stc = st[c * NC_N:(c + 1) * NC_N]      # [32, 16, 16]
        s_st = stc.sum(-1)                     # [32, 16]
        q_st = (stc ** 2).sum(-1)              # [32, 16]
        # static part with its share of the mean term folded in
        sp = (np.einsum("nhp,pd->nhd", stc, w1s)
              - s_st[:, :, None] * cw[None, None, :] / 32.0)   # [32, 16, 64]

        # K-row layout per slab column (r in [0,128)):
        #   r in [0,64):   dyn*inv, a = r//16, f = r%16
        #   r = 64+8a+v':  sel row (value inv iff v'==slab v) -> sp'[n,4v'+a]
        #   r = 96:        const 1 (shared) -> b1p
        #   elsewhere:     zero
        wa = np.zeros((NC_N, 128, 128), np.float32)
        wb = np.zeros((NC_N, 128, 128), np.float32)
        for n in range(NC_N):
            for a in range(4):
                tgt = wa if a < 2 else wb
                mcol = 64 * (a % 2)
                tgt[n, 16 * a:16 * a + 16, mcol:mcol + 64] = w1dp
                for vv in range(4):
                    tgt[n, 64 + 8 * a + vv, mcol:mcol + 64] = sp[n, 4 * vv + a]
            wa[n, 96, 0:64] = b1p
            wa[n, 96, 64:128] = b1p
            wb[n, 96, 0:64] = b1p
            wb[n, 96, 64:128] = b1p

        # chunk-major layout: [e, p, n, h, f] so each chunk load is one
        # contiguous 2 MB stream (bt = 8p + e)
        dyn_cm = (dyn[:, c * NC_N:(c + 1) * NC_N]
                  .reshape(128, 8, NC_N, H, PD).transpose(1, 0, 2, 3, 4))
        sstp = (s_st / 32.0).reshape(512)      # S_st/32 per (n,h)
        qstp = (q_st / 32.0).reshape(512)      # Q_st/32
        per_core.append({
            "dyn": np.ascontiguousarray(dyn_cm).astype(ml_dtypes.bfloat16),
            "wa": np.ascontiguousarray(wa.transpose(1, 0, 2)).astype(ml_dtypes.bfloat16),
            "wb": np.ascontiguousarray(wb.transpose(1, 0, 2)).astype(ml_dtypes.bfloat16),
            "m2": np.ascontiguousarray(m2).astype(ml_dtypes.bfloat16),
            "sst32": np.ascontiguousarray(sstp.reshape(1, 512).astype(np.float32)),
            "qst32": np.ascontiguousarray((qstp + EPS).reshape(1, 512).astype(np.float32)),
        })
    return per_core


def build_nc(n_chunks=8):
    nc = bacc.Bacc("TRN2", target_bir_lowering=False, debug=False, num_devices=NCORES)
    dyn = nc.dram_tensor("dyn", [8, 128, NC_N, H, PD], BF16, kind="ExternalInput")
    wa_d = nc.dram_tensor("wa", [128, NC_N, 128], BF16, kind="ExternalInput")
    wb_d = nc.dram_tensor("wb", [128, NC_N, 128], BF16, kind="ExternalInput")
    m2_d = nc.dram_tensor("m2", [128, 8, 8], BF16, kind="ExternalInput")
    sst_d = nc.dram_tensor("sst32", [1, 512], F32, kind="ExternalInput")
    qst_d = nc.dram_tensor("qst32", [1, 512], F32, kind="ExternalInput")
    # out[e, p, hb, i, (nl j m)] bf16 ; host maps n = hb*16 + j*4 + nl,
    # h = 2m + i and casts to f32.
    out_d = nc.dram_tensor("out", [8, 128, 2, 2, 128], BF16, kind="ExternalOutput")

    dyn_v = dyn[:, :, :, :, :]
    out_v = out_d[:, :, :, :, :]
    NH = 16  # n's per half-chunk

    with TileContext(nc) as tc:
        with tc.tile_pool(name="const", bufs=1) as cpool, \
             tc.tile_pool(name="stg", bufs=2) as stgpool, \
             tc.tile_pool(name="stats", bufs=1) as stpool, \
             tc.tile_pool(name="invp", bufs=2) as invpool, \
             tc.tile_pool(name="tr", bufs=2) as trpool, \
             tc.tile_pool(name="hid", bufs=4) as hpool, \
             tc.tile_pool(name="scb", bufs=2) as scbpool, \
             tc.tile_pool(name="sm", bufs=3) as smpool, \
             tc.tile_pool(name="ex", bufs=2) as expool, \
             tc.tile_pool(name="ot", bufs=2) as otpool, \
             tc.tile_pool(name="ps1", bufs=3, space="PSUM") as ps1pool, \
             tc.tile_pool(name="ps2", bufs=1, space="PSUM") as psm2pool:

            # ---- constants / weights (loaded once, host pre-transposed) ----
            wat = cpool.tile([128, NC_N, 128], BF16)
            nc.sync.dma_start(wat[:, :, :], wa_d[:, :, :])
            wbt = cpool.tile([128, NC_N, 128], BF16)
            nc.sync.dma_start(wbt[:, :, :], wb_d[:, :, :])
            m2t = cpool.tile([128, 8, 8], BF16)
            nc.sync.dma_start(m2t[:, :, :], m2_d[:, :, :])
            sstt = cpool.tile([128, 512], F32)
            nc.sync.dma_start(sstt[0:1, :], sst_d[:, :])
            nc.gpsimd.partition_broadcast(sstt[:, :], sstt[0:1, :], channels=128)
            qstt = cpool.tile([128, 512], F32)
            nc.sync.dma_start(qstt[0:1, :], qst_d[:, :])
            nc.gpsimd.partition_broadcast(qstt[:, :], qstt[0:1, :], channels=128)

            # ---- persistent half-chunk slab buffers (4, rotated) ----
            slabs = []
            for i in range(4):
                sl = cpool.tile([128, NH, 4, 128], BF16, tag=f"slab{i}")
                slf = sl[:, :, :, :].rearrange("p n v c -> p (n v c)")
                p0 = list(slf.ap)[0]
                nc.gpsimd.memset(sl[:, :, :, 64:128], 0.0)
                nc.vector.memset(BassAP(slf.tensor, slf.offset + 96,
                                        [p0, [512, NH], [128, 4]]), 1.0)
                slabs.append(sl)

            def slab_half(b8, hc):
                return slabs[(2 * b8 + hc) % 4]

            stg_tiles = {}
            fill_state = {}

            def load_phase(b8):
                """Issue HBM loads for chunk b8 (2 chunks ahead of compute)."""
                stg = stgpool.tile([128, NC_N, H, PD], BF16, tag="stg")
                stg_tiles[b8] = stg
                for hc in range(2):
                    n0 = hc * NH
                    nc.sync.dma_start(stg[:, n0:n0 + NH, :, :],
                                      dyn_v[b8, :, n0:n0 + NH, :, :])

            def stats_phase(b8):
                """Sum/sumsq via fp16 pairwise TT trees (2x DVE rate) ->
                invstd for chunk b8. Processed per half so the h0 slab
                fill starts after only half the stats latency."""
                stg = stg_tiles.pop(b8)
                ssum = stpool.tile([128, 512], F32, tag="ssum")
                q = stpool.tile([128, 512], F32, tag="q")
                mean = stpool.tile([128, 512], F32, tag="mean")
                vareps = stpool.tile([128, 512], F32, tag="vareps")
                seed = stpool.tile([128, 512], U32, tag="seed")
                tmp = stpool.tile([128, 512], F32, tag="tmp")
                inv = invpool.tile([128, 512], F32, tag="inv")
                inv_nva = inv[:, :].rearrange("p (n v a) -> p n v a", n=NC_N, v=4)
                fill_state[b8] = (stg, inv_nva)
                for hc in range(2):
                    n0 = hc * NH
                    o0, o1 = n0 * 16, (n0 + NH) * 16
                    stg_f = stg[:, n0:n0 + NH, :, :].rearrange("p n h f -> p (n h) f")
                    sqt = stpool.tile([128, 256, PD], FP16, tag="sqt")
                    nc.vector.tensor_tensor(sqt[:, :, :], stg_f, stg_f, AT.mult)
                    for name, src in (("s", stg_f), ("q", sqt[:, :, :])):
                        t8 = stpool.tile([128, 256, 8], FP16, tag=f"{name}t8")
                        nc.vector.tensor_tensor(t8[:, :, :], src[:, :, 0:8], src[:, :, 8:16], AT.add)
                        t4 = stpool.tile([128, 256, 4], FP16, tag=f"{name}t4")
                        nc.vector.tensor_tensor(t4[:, :, :], t8[:, :, 0:4], t8[:, :, 4:8], AT.add)
                        t2 = stpool.tile([128, 256, 2], FP16, tag=f"{name}t2")
                        nc.vector.tensor_tensor(t2[:, :, :], t4[:, :, 0:2], t4[:, :, 2:4], AT.add)
                        dst = (ssum if name == "s" else q)[:, o0:o1]
                        nc.vector.tensor_tensor(dst, t2[:, :, 0], t2[:, :, 1], AT.add)

                    # half-chunk stats chain [128, 256]
                    nc.vector.scalar_tensor_tensor(mean[:, o0:o1], ssum[:, o0:o1], 1.0 / 32, sstt[:, o0:o1], AT.mult, AT.add)
                    nc.vector.scalar_tensor_tensor(vareps[:, o0:o1], q[:, o0:o1], 1.0 / 32, qstt[:, o0:o1], AT.mult, AT.add)
                    nc.vector.scalar_tensor_tensor(mean[:, o0:o1], mean[:, o0:o1], -1.0, mean[:, o0:o1], AT.mult, AT.mult)
                    nc.vector.tensor_tensor(vareps[:, o0:o1], vareps[:, o0:o1], mean[:, o0:o1], AT.add)

                    # invstd via bit-trick rsqrt + 1 Newton step
                    nc.vector.tensor_scalar(seed[:, o0:o1], vareps[:, o0:o1].bitcast(U32), 1, None, AT.logical_shift_right)
                    nc.vector.tensor_scalar(seed[:, o0:o1], seed[:, o0:o1], -1.0, MAGIC, AT.mult, AT.add)
                    y0 = seed[:, o0:o1].bitcast(F32)
                    nc.vector.tensor_tensor(tmp[:, o0:o1], y0, y0, AT.mult)
                    nc.vector.scalar_tensor_tensor(tmp[:, o0:o1], tmp[:, o0:o1], -0.5, vareps[:, o0:o1], AT.mult, AT.mult)
                    nc.vector.tensor_scalar(tmp[:, o0:o1], tmp[:, o0:o1], 1.5, None, AT.add)
                    nc.vector.tensor_tensor(inv[:, o0:o1], y0, tmp[:, o0:o1], AT.mult)
                    if hc == 0:
                        fill_half(b8, 0)

            def fill_half(b8, hc):
                """Write half-slab hc of chunk b8 (sel rows + scaled dyn)."""
                stg, inv_nva = fill_state[b8]
                n0 = hc * NH
                sl = slab_half(b8, hc)
                slf = sl[:, :, :, :].rearrange("p n v c -> p (n v c)")
                p0 = list(slf.ap)[0]
                # sel rows: slab[p, n, v, 64+8a+v] = inv[p, n, v, a]
                nc.vector.tensor_copy(
                    BassAP(slf.tensor, slf.offset + 64,
                           [p0, [512, NH], [129, 4], [8, 4]]),
                    inv_nva[:, n0:n0 + NH, :, :])
                # scale dyn by invstd on GPSIMD (split per gb so each 4-n
                # group's transpose can start as soon as its quarter lands)
                for gb in range(4):
                    g0 = gb * 4
                    inv_h = (inv_nva[:, n0 + g0:n0 + g0 + 4, :, :]
                             .rearrange("p n v (a o) -> p n v a o", o=1)
                             .broadcast_to([128, 4, 4, 4, 16]))
                    nc.gpsimd.tensor_tensor(
                        sl[:, g0:g0 + 4, :, 0:64].rearrange("p n v (a f) -> p n v a f", a=4),
                        stg[:, n0 + g0:n0 + g0 + 4, :, :].rearrange("p n (v a) f -> p n v a f", v=4),
                        inv_h, AT.mult)

            def mm_phase(b8, prev_tail=None):
                """Transpose slab b8, mm1/tanh/mm2-tiled; returns tail state.

                The softmax tail of chunk b8-1 is emitted first: all its
                inputs are a full chunk old, so no engine's FIFO stalls on
                in-flight work of chunk b8."""
                if prev_tail is not None:
                    emit_tail(b8 - 1, prev_tail)
                half_out = []
                for hb in range(2):
                    if hb == 1 and b8 + 1 in fill_state:
                        fill_half(b8 + 1, 1)
                    sl = slab_half(b8, hb)
                    hts = []
                    for gb in range(4):
                        nb = hb * 4 + gb
                        trt = trpool.tile([128, 16, 128], BF16, tag="tr")
                        nc.sync.dma_start_transpose(
                            trt[:, :, :],
                            sl[:, gb * 4:(gb + 1) * 4, :, :].rearrange("p n v c -> p (n v c)"))
                        # [p, nl, half, v, c]
                        ht4 = hpool.tile([128, 4, 2, 4, 128], BF16, tag="h")
                        hts.append(ht4)
                        for nl in range(4):
                            n = nb * 4 + nl
                            ps = ps1pool.tile([128, 1024], F32, tag="ps1")
                            rhs4 = trt[:, nl * 4:nl * 4 + 4, :].rearrange("p s c -> p (s c)")
                            nc.tensor.matmul(ps[:, 0:512], wat[:, n, :], rhs4, start=True, stop=True)
                            nc.tensor.matmul(ps[:, 512:1024], wbt[:, n, :], rhs4, start=True, stop=True)
                            nc.scalar.activation(
                                ht4[:, nl, :, :, :].rearrange("p a b c -> p (a b) c"),
                                ps[:, :].rearrange("p (s c) -> p s c", s=8), AFT.Tanh)
                    # mm2: 8 concurrent 64x8 tiles: (i = token parity -> rows
                    # 64i..; j = gb -> psum partitions 32j..32j+8, bank i).
                    pa = psm2pool.tile([128, 512], F32, tag="m2a")
                    pb = psm2pool.tile([128, 512], F32, tag="m2b")
                    pst = [pa, pb]
                    for s in range(8):
                        half, v = s // 4, s % 4
                        for j in range(4):
                            for i in range(2):
                                nc.tensor.matmul(
                                    pst[i][32 * j:32 * j + 8, :],
                                    m2t[64 * i:64 * i + 64, s, :],
                                    hts[j][64 * i:64 * i + 64, :, half, v, :],
                                    start=(s == 0), stop=(s == 7),
                                    tile_position=(64 * i, 32 * j))
                    # drain: cast to bf16, xbar transpose to token-major
                    scts = []
                    for i in range(2):
                        sc = scbpool.tile([128, 512], BF16, tag=f"sc{i}")
                        nc.vector.tensor_copy(sc[:, :], pst[i][:, :])
                        sct = smpool.tile([128, 4, 128], BF16, tag=f"sct{i}")
                        nc.sync.dma_start_transpose(sct[:, :, :], sc[:, :])
                        scts.append(sct)
                    half_out.append(scts)
                return half_out

            def emit_tail(b8, half_out):
                """Softmax tail: exp, denominators, normalize, DMA out."""
                for hb in range(2):
                    scts = half_out[hb]
                    exs = []
                    for i in range(2):
                        sct_v = (scts[i][:, :, :]
                                 .rearrange("p a (j q) -> p a j q", j=4)[:, :, :, 0:8])
                        ex = expool.tile([128, 4, 4, 8], BF16, tag=f"ex{i}")
                        nc.scalar.activation(ex[:, :, :, :], sct_v, AFT.Exp)
                        exs.append(ex)
                    den0 = smpool.tile([128, 16], F32, tag="den0")
                    nc.vector.tensor_reduce(
                        den0[:, :], exs[0][:, :, :, :].rearrange("p a j m -> p (a j) m"),
                        axis=mybir.AxisListType.X, op=AT.add)
                    den1 = smpool.tile([128, 16], F32, tag="den1")
                    nc.vector.tensor_reduce(
                        den1[:, :], exs[1][:, :, :, :].rearrange("p a j m -> p (a j) m"),
                        axis=mybir.AxisListType.X, op=AT.add)
                    nc.vector.tensor_tensor(den0[:, :], den0[:, :], den1[:, :], AT.add)
                    rcp = smpool.tile([128, 16], F32, tag="rcp")
                    nc.vector.reciprocal(rcp[:, :], den0[:, :])
                    rcp_b = (rcp[:, :].rearrange("p (n o) -> p n o", o=1)
                             .broadcast_to([128, 16, 8]))
                    for i in range(2):
                        ot = otpool.tile([128, 16, 8], BF16, tag=f"ot{i}")
                        nc.vector.tensor_tensor(
                            ot[:, :, :],
                            exs[i][:, :, :, :].rearrange("p a j m -> p (a j) m"),
                            rcp_b, AT.mult)
                        nc.gpsimd.dma_start(out_v[b8, :, hb, i, :],
                                            ot[:, :, :].rearrange("p n m -> p (n m)"))

            # software pipeline: loads lead by 2 chunks, stats by 1.
            load_phase(0)
            if n_chunks > 1:
                load_phase(1)
            stats_phase(0)
            fill_half(0, 1)
            tail = None
            for b8 in range(n_chunks):
                if b8 + 2 < n_chunks:
                    load_phase(b8 + 2)
                if b8 + 1 < n_chunks:
                    stats_phase(b8 + 1)
                tail = mm_phase(b8, tail)
                fill_state.pop(b8, None)
            emit_tail(n_chunks - 1, tail)
    nc.compile()
    return nc


# out slot permutation: device slot (i, m) -> h = 2m + i
_SLOT_OF_H = np.array([(h % 2) * 8 + h // 2 for h in range(16)], np.int64)


def kernel(**inputs):
    per_core = _host_prep(**inputs)
    if "nc" not in _cached:
        _cached["nc"] = build_nc()
    nc = _cached["nc"]
    trace = bool(os.environ.get("DWA_TRACE"))
    res = run_bass_kernel_spmd(nc, per_core, core_ids=list(range(NCORES)), trace=trace)
    if trace:
        print("HW exec time:", res.exec_time_ns, "ns")
        kernel.last_result = res
    out = np.empty((B_T, N, H), np.float32)
    for c in range(NCORES):
        oc = np.asarray(res.results[c]["out"]).astype(np.float32)
        # [8 e, 128 p, 2 hb, 2 i, 256=(nl 4, j 4, m 8)]
        oc = oc.reshape(8, 128, 2, 2, 4, 4, 8)
        # -> [p, e, hb, j, nl, (i, m)]
        oc = oc.transpose(1, 0, 2, 5, 4, 3, 6).reshape(B_T, 2, 4, 4, 16)
        oc = oc.reshape(B_T, NC_N, 16)[:, :, _SLOT_OF_H]
        out[:, c * NC_N:(c + 1) * NC_N, :] = oc
    return out
